# revision 88
# baseline (speedup 1.0000x reference)
"""Trainium2 Bass kernel for nn_Attention_30468497997979.

Reference computation (per batch b of 8):
    X = hidden_states[b,:,0,:]              # (C=768, S=384)
    Q/K/V = W @ X + b                       # 1x1 conv == channel matmul
    per head h (12 heads, head dim 64, channel c = d*12 + h):
        scores = (Q_h^T K_h) / 8, mask q>k, softmax over k
        attn_h = V_h @ softmax
    out = Wo @ concat_heads(attn)           # channel c = h*64 + d
Sharding: pure data-parallel, one batch per NeuronCore (8 cores).

Per-core kernel design (DMA/schedule-optimized; ~67.6us best measured
vs the 98.4us v1 baseline — remaining time is ~6us compute-start
latency, ~37us of PE matmul streaming, and a fixed ~10us NEFF
drain/semaphore-reset epilogue):
  - All matmul data is bf16 (1 PE col/cycle); PSUM accumulation fp32.
  - Host pre-permutes W_{q,k,v} rows to head-major channel order
    (c' = h*64 + d), transposes to [c_in, c_out], folds 1/sqrt(d) into
    Wq/bq, folds the V bias through attention (softmax rows sum to 1)
    into an output bias Wo @ bv.
  - Only the Sync and Activation DMA queues are hardware-dynamic
    (~134 GB/s each); the GpSimd queue is software-dynamic (~13 GB/s)
    and carries only tiny SBUF->SBUF moves. Weights are host-packed so
    every SBUF tile is one contiguous hardware DMA, issued in need
    order per queue: x and wq0 split across both queues first, then
    wk0/wk1, wv, the remaining blocks, wo last. The Activation engine
    also runs the exp pipeline, so its later DMAs are issued from
    mid-program points.
  - scores are computed transposed ([k, q], keys on partitions) into
    two PSUM banks per head (k-chunks 0+2 packed into one 512-col
    bank) -> 2 exps per head instead of 3. No max-subtraction needed
    (scores are O(1)). Causal masking is a post-exp 0/1 multiply on
    the SBUF e-tiles, on GpSimd (the only engine with spare cycles;
    nothing else can run there — GpSimd has no PSUM port).
  - attn@V contracts over k on partitions with a fused ones-column in
    each V tile producing the softmax denominator as PSUM row 64; one
    [65, 384] Activation-engine copy moves attn+denominator to SBUF.
  - A whole normalize-group's denominator rows scatter in ONE DMA to
    [8, 48] blocks per head so the DVE reciprocal runs on 32 full
    partitions per group (~0.4us vs 2.5us on 12 lanes), then a K=1
    all-bf16 PE matmul
    (f32r would force a ~1us PE pipeline mode switch) broadcasts each
    row across 64 partitions for the DVE normalize multiply.
  - The schedule keeps every PSUM ring's producer ~1us of PE work away
    from its consumer: Q/K projections and normalize broadcasts
    interleave with scores/attn@V, normalize groups (4/4/2/2 heads)
    resolve two heads after their last attn@V, and the first two
    output-projection chunks accumulate cc 0..3 before the last
    normalize group's broadcasts so the final chain is hidden.
  - Output is written in bf16 (host upcasts; well within tolerance),
    split across both hardware queues.
"""

import numpy as np

B, C, S, H, D = 8, 768, 384, 12, 64
NC = C // 128  # 6
NEG = -10000.0

_STATE = {}


# --------------------------------------------------------------------------
# Workaround: this walrus build rejects the multi-wait InstDrain that
# TileContext emits at exit ("Too many sync wait commands"). Split the
# drain's sem waits onto standalone sync-engine wait instructions.
def _patch_walrus_flags():
    """Append walrus flags: cap the semaphore space so the compiler's
    end-of-NEFF semaphore-reset epilogue (one instruction per HW sem,
    ~7us for all 256) only covers the sems actually in use."""
    import concourse.bass_utils as bu

    if getattr(bu, "_flags_patch", False):
        return
    orig = bu.run_command

    def patched(argv, **kwargs):
        return orig(argv, **kwargs)

    bu.run_command = patched
    bu._flags_patch = True


def _patch_tile_drain():
    import concourse.tile as tile_mod
    from concourse.vector_clock import ScopedClock
    from bass_rust import SyncInfo

    if getattr(tile_mod.TileContext, "_drain_split_patch", False):
        return

    def _drain_and_barrier_split(self, tick_clock, wait_clock):
        nc = self.nc
        assert self.sems is not None
        handles = {}
        for h in self.sems.allocated().values():
            handles[h.num] = h
            handles[h.name] = h

        probe = nc.sync.nop()
        wait_clock.add_sem_waits(
            probe.ins, ScopedClock({None: tick_clock.global_clock})
        )
        waits = list(probe.ins.sync_info.on_wait)
        probe.ins.sync_info = SyncInfo(on_wait=[], on_update=[])
        for w in waits:
            h = handles.get(w.id) or handles.get(w.ant_name)
            if h is not None:
                nc.sync.wait_ge(h, w.wait_value)
            else:
                n2 = nc.sync.nop()
                n2.ins.sync_info = SyncInfo(on_wait=[w], on_update=[])

        drain_inst = nc.sync.drain()
        wait_clock.add_sem_waits(
            drain_inst.ins, ScopedClock({None: tick_clock.global_clock})
        )
        if list(drain_inst.ins.sync_info.on_wait):
            drain_inst.ins.sync_info = SyncInfo(on_wait=[], on_update=[])

        nc.all_engine_barrier()
        popped = nc._tile_sem_poison_stack.pop()
        assert popped is self._sem_poison
        nc.clear_and_free_semaphores(list(self.sems.allocated().values()))
        nc.all_engine_barrier()

        # This walrus codegen supports at most ONE sem wait per
        # instruction. Move extra waits onto same-engine nop carriers
        # inserted just before the instruction (engine queues execute in
        # order, so the semantics are identical).
        import concourse.mybir as mybir

        k = 0
        for f in nc.m.functions:
            for bb in f.blocks:
                new_insts = []
                for inst in bb.instructions:
                    si = inst.sync_info
                    waits = list(si.on_wait) if si else []
                    if len(waits) > 1:
                        for w in waits[:-1]:
                            nop = mybir.InstNoOp(name=f"I-wsplit-{k}")
                            k += 1
                            nop.engine = inst.engine
                            nop.sync_info = SyncInfo(on_wait=[w], on_update=[])
                            nc.register_instruction(nop)
                            new_insts.append(nop)
                        inst.sync_info = SyncInfo(
                            on_wait=[waits[-1]], on_update=list(si.on_update)
                        )
                    new_insts.append(inst)
                bb.instructions = new_insts

    tile_mod.TileContext._drain_and_barrier = _drain_and_barrier_split
    tile_mod.TileContext._drain_split_patch = True


# --------------------------------------------------------------------------
def _build_nc():
    import concourse.bass as bass
    import concourse.mybir as mybir
    import concourse.tile as tile

    _patch_tile_drain()
    _patch_walrus_flags()

    f32 = mybir.dt.float32
    f32r = mybir.dt.float32r
    bf16 = mybir.dt.bfloat16
    Ident = mybir.ActivationFunctionType.Identity
    Exp = mybir.ActivationFunctionType.Exp

    nc = bass.Bass()
    # host-packed inputs (see _prep_maps for the exact layouts)
    x_d = nc.dram_tensor("xp", [128, NC * S], bf16, kind="ExternalInput")
    wq_d = nc.dram_tensor("wqp", [NC, 128, C], bf16, kind="ExternalInput")
    wk_d = nc.dram_tensor("wkp", [NC, 128, C], bf16, kind="ExternalInput")
    wo_d = nc.dram_tensor("wop", [NC, 128, C], bf16, kind="ExternalInput")
    wv_d = nc.dram_tensor("wvp", [128, NC * C], bf16, kind="ExternalInput")
    # consts: cols 0:6 bq, 6:12 bk, 12:18 obias, 18:82 the 0/1 bf16
    # causal mask block (mb[k, q] = 1 where k >= q) bitcast into f32
    cb_d = nc.dram_tensor("cb", [128, 82], f32, kind="ExternalInput")
    # bf16 output halves writeback bytes; host upcasts to f32
    y_d = nc.dram_tensor("y", [C, S], bf16, kind="ExternalOutput")

    with tile.TileContext(nc) as tc:
        with (
            tc.tile_pool(name="persist", bufs=1) as persist,
            tc.tile_pool(name="epool", bufs=6) as epool,
            tc.tile_pool(name="opool", bufs=3) as opool,
            tc.tile_pool(name="psP", bufs=2, space="PSUM") as psP,
            tc.tile_pool(name="psS", bufs=3, space="PSUM") as psS,
            tc.tile_pool(name="psV", bufs=1, space="PSUM") as psV,
            tc.tile_pool(name="psR", bufs=2, space="PSUM") as psR,
        ):
            # ---- persistent tiles -------------------------------------
            xt = persist.tile([128, NC, S], bf16, tag="x", name="x")
            wvb = persist.tile([128, NC, C], bf16, tag="wv", name="wv")
            wqall = persist.tile([128, NC, NC, 128], bf16, tag="wq", name="wq")
            wqb = [wqall[:, i] for i in range(NC)]
            wkb = [
                persist.tile([128, NC, 128], bf16, tag=f"wk{i}", name=f"wk{i}")
                for i in range(NC)
            ]
            wob = [
                persist.tile([128, NC, 128], bf16, tag=f"wo{i}", name=f"wo{i}")
                for i in range(NC)
            ]
            cb = persist.tile([128, 82], f32, tag="cb", name="cb")
            mb = cb[:, 18:82].bitcast(bf16)
            q_sb = [
                persist.tile([128, S], bf16, tag=f"q{i}", name=f"q{i}")
                for i in range(NC)
            ]
            k_sb = [
                persist.tile([128, S], bf16, tag=f"k{i}", name=f"k{i}")
                for i in range(NC)
            ]
            vt = [
                persist.tile([128, H, D + 1], bf16, tag=f"vt{sq}", name=f"vt{sq}")
                for sq in range(3)
            ]
            # one shared unnormalized-attn tile: [65 part, head, q] so a
            # whole group's denominator rows move in ONE DMA
            au = persist.tile([D + 1, H, S], f32, tag="au", name="au")
            attn_sb = [
                persist.tile([128, S], bf16, tag=f"at{i}", name=f"at{i}")
                for i in range(NC)
            ]
            # head h's denominator row lives at partitions
            # base(h) .. base(h)+8, 48 q-values per partition
            sums_sb = persist.tile([128, 48], f32, tag="sums", name="sums")
            # 1/sum in bf16: keeps the broadcast matmul all-bf16 (a
            # f32r matmul here forces a PE pipeline mode switch costing
            # ~0.5-1.2us per normalize broadcast)
            rinv_sb = persist.tile([128, 48], bf16, tag="rinv", name="rinv")
            rinv_r = persist.tile([1, H, S], bf16, tag="rinvr", name="rinvr")
            ones_sb = persist.tile([1, D], bf16, tag="ones", name="ones")

            # ---- DMA issue --------------------------------------------
            # Only the Sync and Activation queues are hardware-dynamic
            # (~134 GB/s each); the GpSimd queue is software-dynamic at
            # ~13 GB/s aggregate, so it only carries the tiny SBUF->SBUF
            # denominator moves. Per-queue order = priority (need order).
            # The Activation engine also runs the proj-copy/exp pipeline,
            # so only its startup-critical DMAs are issued up front; the
            # rest are issued from mid-program points (the queue keeps
            # streaming earlier transfers meanwhile).
            nc.scalar.dma_start(xt[:, 0:3, :], x_d[:, 0 : 3 * S])
            nc.scalar.dma_start(wqall[:, 0, 0:3, :], wq_d[0][:, 0 : 3 * 128])
            nc.scalar.dma_start(cb[:], cb_d[:, :])
            nc.sync.dma_start(xt[:, 3:6, :], x_d[:, 3 * S : 6 * S])
            nc.sync.dma_start(wqall[:, 0, 3:6, :], wq_d[0][:, 3 * 128 : C])
            nc.sync.dma_start(wkb[0][:], wk_d[0])
            nc.sync.dma_start(wkb[1][:], wk_d[1])
            nc.sync.dma_start(wvb[:, 3:6, :], wv_d[:, 3 * C : 6 * C])
            for i in range(2, NC):
                nc.sync.dma_start(wkb[i][:], wk_d[i])
            for i in range(NC):
                nc.sync.dma_start(wob[i][:], wo_d[i])

            nc.vector.memset(ones_sb[:], 1.0)
            for sq in range(3):
                nc.vector.memset(vt[sq][:, :, D : D + 1], 1.0)

            # ---- building blocks --------------------------------------
            def qk_proj(oc, wtiles, bias_col, out, on_scalar):
                # copy+bias: Q chunks on Scalar (activation+bias), K
                # chunks on DVE (tensor_scalar add) — splits the
                # PSUM-evacuation load across both engines
                ps = psP.tile([128, S], f32, tag="proj", name="proj")
                for cc in range(NC):
                    nc.tensor.matmul(
                        ps[:],
                        wtiles[oc][:, cc, :],
                        xt[:, cc, :],
                        start=(cc == 0),
                        stop=(cc == NC - 1),
                    )
                if on_scalar:
                    nc.scalar.activation(
                        out[:], ps[:], Ident, bias=cb[:, bias_col : bias_col + 1]
                    )
                else:
                    nc.vector.tensor_scalar_add(
                        out[:], ps[:], cb[:, bias_col : bias_col + 1]
                    )

            def v_proj_unit(half, sq):
                # vt[sq][:, half*6:(half+1)*6, 0:64] = (X^T Wv')[s, c' half]
                # cc order follows wv chunk DMA arrival (sync half 3:6
                # lands first, then the deferred scalar half 0:3)
                cc_order = [0, 1, 2, 3, 4, 5]
                ps = psP.tile([128, S], f32, tag="proj", name="vp")
                for step, cc in enumerate(cc_order):
                    nc.tensor.matmul(
                        ps[:],
                        xt[:, cc, sq * 128 : (sq + 1) * 128],
                        wvb[:, cc, half * 384 : (half + 1) * 384],
                        start=(step == 0),
                        stop=(step == NC - 1),
                    )
                nc.vector.tensor_copy(
                    vt[sq][:, half * 6 : (half + 1) * 6, 0:D],
                    ps[:].rearrange("p (h d) -> p h d", d=D),
                )

            def head_scores(h):
                # A = [kc0 (q 0:128) | kc2 (q 0:384)], B = [kc1 (q 0:256)]
                # B runs first so its exp clears the PSUM ring early.
                # Causal masking happens post-exp as a 0/1 multiply on the
                # SBUF e-tiles (exp(-1e4)==0 == exp(s)*0), which keeps the
                # PSUM->exp chain short and runs on the idle engines.
                oc, prow = h // 2, (h % 2) * D
                Qh = q_sb[oc][prow : prow + D, :]
                Kh = k_sb[oc][prow : prow + D, :]
                ps_b = psS.tile([128, 512], f32, tag="s", name="sb")
                ps_a = psS.tile([128, 512], f32, tag="s", name="sa")
                nc.tensor.matmul(
                    ps_b[:, 0:256], Kh[:, 128:256], Qh[:, 0:256],
                    start=True, stop=True,
                )
                eB = epool.tile([128, 256], bf16, tag="eB", name="eB")
                nc.scalar.activation(eB[:], ps_b[:, 0:256], Exp)
                nc.gpsimd.tensor_mul(eB[:, 128:256], eB[:, 128:256], mb[:])
                nc.tensor.matmul(
                    ps_a[:, 0:128], Kh[:, 0:128], Qh[:, 0:128],
                    start=True, stop=True,
                )
                nc.tensor.matmul(
                    ps_a[:, 128:512], Kh[:, 256:384], Qh[:, 0:384],
                    start=True, stop=True, skip_group_check=True,
                )
                eA = epool.tile([128, 512], bf16, tag="eA", name="eA")
                nc.scalar.activation(eA[:], ps_a[:], Exp)
                # one strided op masks both diagonal blocks (cols 0:128
                # and 384:512 = stride-3 pairs of 128-col groups)
                eAv = eA[:].rearrange("p (a b) -> p a b", b=128)[:, ::3, :]
                _, mbb = bass.broadcast_tensor_aps(
                    eAv, mb.rearrange("p (a b) -> p a b", a=1)
                )
                nc.gpsimd.tensor_mul(eAv, eAv, mbb)
                return eA, eB

            def head_av(h, eA, eB):
                # attn@V with fused denominator column; accumulate widest
                # first so every element's first write carries start
                ps_av = psV.tile([D + 1, S], f32, tag="av", name="av")
                nc.tensor.matmul(
                    ps_av[:, 0:384], vt[2][:, h, :], eA[:, 128:512],
                    start=True, stop=False, skip_group_check=True,
                )
                nc.tensor.matmul(
                    ps_av[:, 0:256], vt[1][:, h, :], eB[:, 0:256],
                    start=False, stop=False, skip_group_check=True,
                )
                nc.tensor.matmul(
                    ps_av[:, 0:128], vt[0][:, h, :], eA[:, 0:128],
                    start=False, stop=True, skip_group_check=True,
                )
                nc.scalar.copy(au[:, h, :], ps_av[:])

            # normalize groups: heads (0-3), (4-7), (8-9), (10-11) at
            # partition bases 0/32/64/96 (compute ops need 32-aligned
            # partition starts)
            GRP = [range(0, 4), range(4, 8), range(8, 10), range(10, 12)]

            def norm_pre(g):
                # one DMA scatters the whole group's denominator rows to
                # [8 partitions x 48] blocks, then a full-partition
                # reciprocal and one repack DMA feed the broadcasts
                hs = GRP[g]
                p0, np_ = 32 * g, 8 * len(hs)
                nc.gpsimd.dma_start(
                    sums_sb[p0 : p0 + np_, :],
                    au[D : D + 1, hs[0] : hs[0] + len(hs), :],
                )
                with nc.allow_low_precision(
                    reason="1/softmax-denom in bf16: denom is O(1-20), "
                    "bf16 keeps ~0.4% relative error, well within budget"
                ):
                    nc.vector.reciprocal(
                        rinv_sb[p0 : p0 + np_, :], sums_sb[p0 : p0 + np_, :]
                    )
                nc.gpsimd.dma_start(
                    rinv_r[0:1, hs[0] : hs[0] + len(hs), :],
                    rinv_sb[p0 : p0 + np_, :],
                )

            def norm_bcast(h):
                # K=1 matmul broadcasts 1/sum across 64 partitions, then
                # the DVE multiply writes the normalized bf16 attn chunk
                oc, prow = h // 2, (h % 2) * D
                ps_r = psR.tile([D, S], f32, tag="rb", name="rb")
                nc.tensor.matmul(
                    ps_r[:], ones_sb[:], rinv_r[0:1, h, :],
                    start=True, stop=True,
                )
                nc.vector.tensor_mul(
                    attn_sb[oc][prow : prow + D, :], au[0:D, h, :], ps_r[:]
                )

            # ---- schedule ---------------------------------------------
            # Q/K projections and scores run ahead while wv streams in;
            # attn@V starts once the V projection lands. Remaining weight
            # DMAs issue from mid-program so the Activation engine's FIFO
            # stays responsive for the proj-copy/exp pipeline. Normalize
            # broadcasts interleave with the next chunk's matmuls to hide
            # the denominator chain (copy->DMA->recip->DMA) latency.
            # psP is single-buffered: q/k projections alternate with
            # scores so the ring never waits on its evacuation copy.
            # psV likewise gets ~1us of independent PE work between
            # attn@V calls.
            es = {}
            qk_proj(0, wqb, 0, q_sb[0], True)
            nc.scalar.dma_start(wqb[1][:], wq_d[1])
            qk_proj(0, wkb, 6, k_sb[0], False)
            es[0] = head_scores(0)
            nc.scalar.dma_start(
                wqall[:, 2:4], wq_d[2:4].rearrange("o p c -> p o c")
            )
            es[1] = head_scores(1)
            qk_proj(1, wqb, 1, q_sb[1], True)
            nc.scalar.dma_start(wvb[:, 0:3, :], wv_d[:, 0 : 3 * C])
            qk_proj(1, wkb, 7, k_sb[1], False)
            nc.scalar.dma_start(
                wqall[:, 4:6], wq_d[4:6].rearrange("o p c -> p o c")
            )
            es[2] = head_scores(2)
            es[3] = head_scores(3)
            qk_proj(2, wqb, 2, q_sb[2], True)
            qk_proj(2, wkb, 8, k_sb[2], False)
            es[4] = head_scores(4)
            es[5] = head_scores(5)
            for sq in range(3):
                v_proj_unit(0, sq)
            for sq in range(3):
                v_proj_unit(1, sq)
            # psV is single-buffered: each attn@V is followed by ~1us of
            # independent PE work so the next one never waits on the
            # PSUM evacuation copy
            head_av(0, *es.pop(0))
            qk_proj(3, wqb, 3, q_sb[3], True)
            head_av(1, *es.pop(1))
            qk_proj(3, wkb, 9, k_sb[3], False)
            head_av(2, *es.pop(2))
            es[6] = head_scores(6)
            head_av(3, *es.pop(3))
            es[7] = head_scores(7)
            norm_pre(0)
            head_av(4, *es.pop(4))
            qk_proj(4, wqb, 4, q_sb[4], True)
            head_av(5, *es.pop(5))
            qk_proj(4, wkb, 10, k_sb[4], False)
            norm_bcast(0)
            norm_bcast(1)
            es[8] = head_scores(8)
            norm_bcast(2)
            norm_bcast(3)
            es[9] = head_scores(9)
            head_av(6, *es.pop(6))
            qk_proj(5, wqb, 5, q_sb[5], True)
            head_av(7, *es.pop(7))
            qk_proj(5, wkb, 11, k_sb[5], False)
            # ---- output projection (bias = host-folded Wo @ bv) -------
            # Partial accumulations over cc 0..3 (which need only the
            # already-normalized heads 0-7) fill the PE gaps where the
            # h8-11 exp and normalize chains would otherwise stall it;
            # chunks 2-4 borrow the (now idle) scores PSUM ring. The
            # cc=4,5 steps follow the group-2/3 multiplies.
            def out_proj_mm(ps, oc, ccs, start, stop):
                for i, cc in enumerate(ccs):
                    nc.tensor.matmul(
                        ps[:],
                        wob[oc][:, cc, :],
                        attn_sb[cc][:],
                        start=(start and i == 0),
                        stop=(stop and i == len(ccs) - 1),
                    )

            def out_proj_fin(ps, oc):
                ot = opool.tile([128, S], bf16, tag="o", name="o")
                nc.scalar.activation(
                    ot[:], ps[:], Ident, bias=cb[:, 12 + oc : 13 + oc]
                )
                eng = nc.sync if oc % 2 == 0 else nc.scalar
                eng.dma_start(y_d[oc * 128 : (oc + 1) * 128, :], ot[:])

            norm_pre(1)
            head_av(8, *es.pop(8))
            es[10] = head_scores(10)
            head_av(9, *es.pop(9))
            es[11] = head_scores(11)
            norm_pre(2)
            ps0 = psP.tile([128, S], f32, tag="proj", name="op")
            out_proj_mm(ps0, 0, [0, 1], True, False)
            head_av(10, *es.pop(10))
            norm_bcast(4)
            norm_bcast(5)
            ps1 = psP.tile([128, S], f32, tag="proj", name="op")
            out_proj_mm(ps1, 1, [0, 1], True, False)
            head_av(11, *es.pop(11))
            norm_pre(3)
            norm_bcast(6)
            norm_bcast(7)
            out_proj_mm(ps0, 0, [2, 3], False, False)
            out_proj_mm(ps1, 1, [2, 3], False, False)
            ps2 = psS.tile([128, 512], f32, tag="s", name="op")[:, 0:S]
            out_proj_mm(ps2, 2, range(4), True, False)
            norm_bcast(8)
            norm_bcast(9)
            ps3 = psS.tile([128, 512], f32, tag="s", name="op")[:, 0:S]
            out_proj_mm(ps3, 3, range(4), True, False)
            norm_bcast(10)
            norm_bcast(11)
            ps4 = psS.tile([128, 512], f32, tag="s", name="op")[:, 0:S]
            out_proj_mm(ps4, 4, range(4), True, False)
            # all cc=4 steps (gated only by group 2's multiplies) run
            # before any cc=5 step so the group-3 multiply latency is
            # hidden behind them
            chunks = ((0, ps0), (1, ps1), (2, ps2), (3, ps3), (4, ps4))
            for oc, ps in chunks:
                out_proj_mm(ps, oc, [4], False, False)
            for oc, ps in chunks:
                out_proj_mm(ps, oc, [5], False, True)
                out_proj_fin(ps, oc)
            ps5 = psP.tile([128, S], f32, tag="proj", name="op")
            out_proj_mm(ps5, 5, range(NC), True, True)
            out_proj_fin(ps5, 5)

    return nc


def _get_nc():
    if "nc" not in _STATE:
        _STATE["nc"] = _build_nc()
    return _STATE["nc"]


# --------------------------------------------------------------------------
def _prep_maps(inputs):
    import ml_dtypes

    bf16 = ml_dtypes.bfloat16
    hs = np.asarray(inputs["hidden_states"], dtype=np.float32)
    Wq = np.asarray(inputs["Wq"], dtype=np.float32)
    bq = np.asarray(inputs["bq"], dtype=np.float32)
    Wk = np.asarray(inputs["Wk"], dtype=np.float32)
    bk = np.asarray(inputs["bk"], dtype=np.float32)
    Wv = np.asarray(inputs["Wv"], dtype=np.float32)
    bv = np.asarray(inputs["bv"], dtype=np.float32)
    Wo = np.asarray(inputs["Wo"], dtype=np.float32)

    # head-major channel permutation: c' = h*64 + d  <-  c = d*12 + h
    idx = (np.arange(H)[:, None] + np.arange(D)[None, :] * H).reshape(C)
    scale = float(D) ** -0.5

    def pack_blocks(wt):
        # wt: [c_in, c_out] -> [oc, p, cc*128 + co]
        w4 = wt.reshape(NC, 128, NC, 128).transpose(2, 1, 0, 3)
        return np.ascontiguousarray(w4.reshape(NC, 128, C)).astype(bf16)

    wqp = pack_blocks((scale * Wq[idx, :]).T)
    wkp = pack_blocks(Wk[idx, :].T)
    wop = pack_blocks(Wo.T)
    # wv: [p, cc*768 + co]
    wvp = np.ascontiguousarray(
        Wv[idx, :].T.reshape(NC, 128, C).transpose(1, 0, 2).reshape(128, NC * C)
    ).astype(bf16)

    cbm = np.zeros((128, 82), dtype=np.float32)
    cbm[:, 0:6] = (scale * bq[idx]).reshape(6, 128).T
    cbm[:, 6:12] = bk[idx].reshape(6, 128).T
    # V-bias folded through attention (softmax rows sum to 1):
    # attn' = attn_nobias' + bv[idx], so out += Wo @ bv[idx]
    cbm[:, 12:18] = (Wo @ bv[idx]).reshape(6, 128).T
    # post-exp causal mask for a diagonal 128-block (keep k >= q),
    # bf16 0/1 values bitcast into the f32 consts tensor
    mbm = np.tril(np.ones((128, 128), dtype=np.float32)).astype(bf16)
    cbm[:, 18:82] = np.ascontiguousarray(mbm).view(np.float32)

    shared = {"wqp": wqp, "wkp": wkp, "wop": wop, "wvp": wvp, "cb": cbm}
    maps = []
    for b in range(B):
        xb = hs[b, :, 0, :].reshape(NC, 128, S).transpose(1, 0, 2)
        xp = np.ascontiguousarray(xb.reshape(128, NC * S)).astype(bf16)
        maps.append({"xp": xp, **shared})
    return maps


def _run(inputs, trace=False, **kwargs):
    from concourse.bass_utils import run_bass_kernel_spmd

    nc = _get_nc()
    in_maps = _prep_maps(inputs)
    res = run_bass_kernel_spmd(
        nc, in_maps, core_ids=list(range(B)), trace=trace, **kwargs
    )
    out = np.stack(
        [res.results[b]["y"].astype(np.float32) for b in range(B)], axis=0
    )
    return out.reshape(B, C, 1, S), res


def kernel(**inputs):
    out, _ = _run(inputs, trace=False)
    return out


# revision 89
# speedup vs baseline: 1.1513x; 1.1513x over previous
"""Trainium2 Bass kernel for nn_Attention_30468497997979.

Reference computation (per batch b of 8):
    X = hidden_states[b,:,0,:]              # (C=768, S=384)
    Q/K/V = W @ X + b                       # 1x1 conv == channel matmul
    per head h (12 heads, head dim 64, channel c = d*12 + h):
        scores = (Q_h^T K_h) / 8, mask q>k, softmax over k
        attn_h = V_h @ softmax
    out = Wo @ concat_heads(attn)           # channel c = h*64 + d
Sharding: pure data-parallel, one batch per NeuronCore (8 cores).

Per-core kernel design (DMA/schedule-optimized; ~67.6us best measured
vs the 98.4us v1 baseline — remaining time is ~6us compute-start
latency, ~37us of PE matmul streaming, and a fixed ~10us NEFF
drain/semaphore-reset epilogue):
  - All matmul data is bf16 (1 PE col/cycle); PSUM accumulation fp32.
  - Host pre-permutes W_{q,k,v} rows to head-major channel order
    (c' = h*64 + d), transposes to [c_in, c_out], folds 1/sqrt(d) into
    Wq/bq, folds the V bias through attention (softmax rows sum to 1)
    into an output bias Wo @ bv.
  - Only the Sync and Activation DMA queues are hardware-dynamic
    (~134 GB/s each); the GpSimd queue is software-dynamic (~13 GB/s)
    and carries only tiny SBUF->SBUF moves. Weights are host-packed so
    every SBUF tile is one contiguous hardware DMA, issued in need
    order per queue: x and wq0 split across both queues first, then
    wk0/wk1, wv, the remaining blocks, wo last. The Activation engine
    also runs the exp pipeline, so its later DMAs are issued from
    mid-program points.
  - scores are computed transposed ([k, q], keys on partitions) into
    two PSUM banks per head (k-chunks 0+2 packed into one 512-col
    bank) -> 2 exps per head instead of 3. No max-subtraction needed
    (scores are O(1)). Causal masking is a post-exp 0/1 multiply on
    the SBUF e-tiles, on GpSimd (the only engine with spare cycles;
    nothing else can run there — GpSimd has no PSUM port).
  - attn@V contracts over k on partitions with a fused ones-column in
    each V tile producing the softmax denominator as PSUM row 64; one
    [65, 384] Activation-engine copy moves attn+denominator to SBUF.
  - A whole normalize-group's denominator rows scatter in ONE DMA to
    [8, 48] blocks per head so the DVE reciprocal runs on 32 full
    partitions per group (~0.4us vs 2.5us on 12 lanes), then a K=1
    all-bf16 PE matmul
    (f32r would force a ~1us PE pipeline mode switch) broadcasts each
    row across 64 partitions for the DVE normalize multiply.
  - The schedule keeps every PSUM ring's producer ~1us of PE work away
    from its consumer: Q/K projections and normalize broadcasts
    interleave with scores/attn@V, normalize groups (4/4/2/2 heads)
    resolve two heads after their last attn@V, and the first two
    output-projection chunks accumulate cc 0..3 before the last
    normalize group's broadcasts so the final chain is hidden.
  - Output is written in bf16 (host upcasts; well within tolerance),
    split across both hardware queues.
"""

import numpy as np

B, C, S, H, D = 8, 768, 384, 12, 64
NC = C // 128  # 6
NEG = -10000.0

_STATE = {}


# --------------------------------------------------------------------------
# Workaround: this walrus build rejects the multi-wait InstDrain that
# TileContext emits at exit ("Too many sync wait commands"). Split the
# drain's sem waits onto standalone sync-engine wait instructions.
def _patch_walrus_flags():
    """Append walrus flags: cap the semaphore space so the compiler's
    end-of-NEFF semaphore-reset epilogue (one instruction per HW sem,
    ~7us for all 256) only covers the sems actually in use."""
    import concourse.bass_utils as bu

    if getattr(bu, "_flags_patch", False):
        return
    orig = bu.run_command

    def patched(argv, **kwargs):
        return orig(argv, **kwargs)

    bu.run_command = patched
    bu._flags_patch = True


def _patch_tile_drain():
    import concourse.tile as tile_mod
    from concourse.vector_clock import ScopedClock
    from bass_rust import SyncInfo

    if getattr(tile_mod.TileContext, "_drain_split_patch", False):
        return

    def _drain_and_barrier_split(self, tick_clock, wait_clock):
        nc = self.nc
        assert self.sems is not None
        handles = {}
        for h in self.sems.allocated().values():
            handles[h.num] = h
            handles[h.name] = h

        probe = nc.sync.nop()
        wait_clock.add_sem_waits(
            probe.ins, ScopedClock({None: tick_clock.global_clock})
        )
        waits = list(probe.ins.sync_info.on_wait)
        probe.ins.sync_info = SyncInfo(on_wait=[], on_update=[])
        for w in waits:
            h = handles.get(w.id) or handles.get(w.ant_name)
            if h is not None:
                nc.sync.wait_ge(h, w.wait_value)
            else:
                n2 = nc.sync.nop()
                n2.ins.sync_info = SyncInfo(on_wait=[w], on_update=[])

        drain_inst = nc.sync.drain()
        wait_clock.add_sem_waits(
            drain_inst.ins, ScopedClock({None: tick_clock.global_clock})
        )
        if list(drain_inst.ins.sync_info.on_wait):
            drain_inst.ins.sync_info = SyncInfo(on_wait=[], on_update=[])

        nc.all_engine_barrier()
        popped = nc._tile_sem_poison_stack.pop()
        assert popped is self._sem_poison
        nc.clear_and_free_semaphores(list(self.sems.allocated().values()))
        nc.all_engine_barrier()

        # This walrus codegen supports at most ONE sem wait per
        # instruction. Move extra waits onto same-engine nop carriers
        # inserted just before the instruction (engine queues execute in
        # order, so the semantics are identical).
        import concourse.mybir as mybir

        k = 0
        for f in nc.m.functions:
            for bb in f.blocks:
                new_insts = []
                for inst in bb.instructions:
                    si = inst.sync_info
                    waits = list(si.on_wait) if si else []
                    if len(waits) > 1:
                        for w in waits[:-1]:
                            nop = mybir.InstNoOp(name=f"I-wsplit-{k}")
                            k += 1
                            nop.engine = inst.engine
                            nop.sync_info = SyncInfo(on_wait=[w], on_update=[])
                            nc.register_instruction(nop)
                            new_insts.append(nop)
                        inst.sync_info = SyncInfo(
                            on_wait=[waits[-1]], on_update=list(si.on_update)
                        )
                    new_insts.append(inst)
                bb.instructions = new_insts

    tile_mod.TileContext._drain_and_barrier = _drain_and_barrier_split
    tile_mod.TileContext._drain_split_patch = True


# --------------------------------------------------------------------------
def _build_nc():
    import concourse.bass as bass
    import concourse.mybir as mybir
    import concourse.tile as tile

    _patch_tile_drain()
    _patch_walrus_flags()

    f32 = mybir.dt.float32
    f32r = mybir.dt.float32r
    bf16 = mybir.dt.bfloat16
    Ident = mybir.ActivationFunctionType.Identity
    Exp = mybir.ActivationFunctionType.Exp

    nc = bass.Bass()
    # host-packed inputs (see _prep_maps for the exact layouts)
    x_d = nc.dram_tensor("xp", [128, NC * S], bf16, kind="ExternalInput")
    wq_d = nc.dram_tensor("wqp", [NC, 128, C], bf16, kind="ExternalInput")
    wk_d = nc.dram_tensor("wkp", [NC, 128, C], bf16, kind="ExternalInput")
    wo_d = nc.dram_tensor("wop", [NC, 128, C], bf16, kind="ExternalInput")
    wv_d = nc.dram_tensor("wvp", [128, NC * C], bf16, kind="ExternalInput")
    # consts: cols 0:6 bq, 6:12 bk, 12:18 obias, 18:82 the 0/1 bf16
    # causal mask block (mb[k, q] = 1 where k >= q) bitcast into f32
    cb_d = nc.dram_tensor("cb", [128, 82], f32, kind="ExternalInput")
    # bf16 output halves writeback bytes; host upcasts to f32
    y_d = nc.dram_tensor("y", [C, S], bf16, kind="ExternalOutput")

    with tile.TileContext(nc) as tc:
        with (
            tc.tile_pool(name="persist", bufs=1) as persist,
            tc.tile_pool(name="epool", bufs=6) as epool,
            tc.tile_pool(name="opool", bufs=3) as opool,
            tc.tile_pool(name="psP", bufs=2, space="PSUM") as psP,
            tc.tile_pool(name="psS", bufs=3, space="PSUM") as psS,
            tc.tile_pool(name="psV", bufs=1, space="PSUM") as psV,
            tc.tile_pool(name="psR", bufs=2, space="PSUM") as psR,
        ):
            # ---- persistent tiles -------------------------------------
            xt = persist.tile([128, NC, S], bf16, tag="x", name="x")
            wvb = persist.tile([128, NC, C], bf16, tag="wv", name="wv")
            wqall = persist.tile([128, NC, NC, 128], bf16, tag="wq", name="wq")
            wqb = [wqall[:, i] for i in range(NC)]
            wkb = [
                persist.tile([128, NC, 128], bf16, tag=f"wk{i}", name=f"wk{i}")
                for i in range(NC)
            ]
            wob = [
                persist.tile([128, NC, 128], bf16, tag=f"wo{i}", name=f"wo{i}")
                for i in range(NC)
            ]
            cb = persist.tile([128, 82], f32, tag="cb", name="cb")
            mb = cb[:, 18:82].bitcast(bf16)
            q_sb = [
                persist.tile([128, S], bf16, tag=f"q{i}", name=f"q{i}")
                for i in range(NC)
            ]
            k_sb = [
                persist.tile([128, S], bf16, tag=f"k{i}", name=f"k{i}")
                for i in range(NC)
            ]
            vt = [
                persist.tile([128, H, D + 1], bf16, tag=f"vt{sq}", name=f"vt{sq}")
                for sq in range(3)
            ]
            # one shared unnormalized-attn tile: [65 part, head, q] so a
            # whole group's denominator rows move in ONE DMA
            au = persist.tile([D + 1, H, S], f32, tag="au", name="au")
            attn_sb = [
                persist.tile([128, S], bf16, tag=f"at{i}", name=f"at{i}")
                for i in range(NC)
            ]
            # head h's denominator row lives at partitions
            # base(h) .. base(h)+8, 48 q-values per partition
            sums_sb = persist.tile([128, 48], f32, tag="sums", name="sums")
            # 1/sum in bf16: keeps the broadcast matmul all-bf16 (a
            # f32r matmul here forces a PE pipeline mode switch costing
            # ~0.5-1.2us per normalize broadcast)
            rinv_sb = persist.tile([128, 48], bf16, tag="rinv", name="rinv")
            rinv_r = persist.tile([1, H, S], bf16, tag="rinvr", name="rinvr")
            ones_sb = persist.tile([1, D], bf16, tag="ones", name="ones")

            # ---- DMA issue --------------------------------------------
            # Only the Sync and Activation queues are hardware-dynamic
            # (~134 GB/s each); the GpSimd queue is software-dynamic at
            # ~13 GB/s aggregate, so it only carries the tiny SBUF->SBUF
            # denominator moves. Per-queue order = priority (need order).
            # The Activation engine also runs the proj-copy/exp pipeline,
            # so only its startup-critical DMAs are issued up front; the
            # rest are issued from mid-program points (the queue keeps
            # streaming earlier transfers meanwhile).
            nc.scalar.dma_start(xt[:, 0:3, :], x_d[:, 0 : 3 * S])
            nc.scalar.dma_start(wqall[:, 0, 0:3, :], wq_d[0][:, 0 : 3 * 128])
            nc.scalar.dma_start(cb[:], cb_d[:, :])
            nc.sync.dma_start(xt[:, 3:6, :], x_d[:, 3 * S : 6 * S])
            nc.sync.dma_start(wqall[:, 0, 3:6, :], wq_d[0][:, 3 * 128 : C])
            nc.sync.dma_start(wkb[0][:], wk_d[0])
            nc.sync.dma_start(wkb[1][:], wk_d[1])
            nc.sync.dma_start(wvb[:, 3:6, :], wv_d[:, 3 * C : 6 * C])
            for i in range(2, NC):
                nc.sync.dma_start(wkb[i][:], wk_d[i])
            for i in range(NC):
                nc.sync.dma_start(wob[i][:], wo_d[i])

            nc.vector.memset(ones_sb[:], 1.0)
            for sq in range(3):
                nc.vector.memset(vt[sq][:, :, D : D + 1], 1.0)

            # ---- building blocks --------------------------------------
            def qk_proj(oc, wtiles, bias_col, out, on_scalar):
                # copy+bias: Q chunks on Scalar (activation+bias), K
                # chunks on DVE (tensor_scalar add) — splits the
                # PSUM-evacuation load across both engines
                ps = psP.tile([128, S], f32, tag="proj", name="proj")
                for cc in range(NC):
                    nc.tensor.matmul(
                        ps[:],
                        wtiles[oc][:, cc, :],
                        xt[:, cc, :],
                        start=(cc == 0),
                        stop=(cc == NC - 1),
                    )
                if on_scalar:
                    nc.scalar.activation(
                        out[:], ps[:], Ident, bias=cb[:, bias_col : bias_col + 1]
                    )
                else:
                    nc.vector.tensor_scalar_add(
                        out[:], ps[:], cb[:, bias_col : bias_col + 1]
                    )

            def v_proj_unit(half, sq):
                # vt[sq][:, half*6:(half+1)*6, 0:64] = (X^T Wv')[s, c' half]
                # cc order follows wv chunk DMA arrival (sync half 3:6
                # lands first, then the deferred scalar half 0:3)
                cc_order = [0, 1, 2, 3, 4, 5]
                ps = psP.tile([128, S], f32, tag="proj", name="vp")
                for step, cc in enumerate(cc_order):
                    nc.tensor.matmul(
                        ps[:],
                        xt[:, cc, sq * 128 : (sq + 1) * 128],
                        wvb[:, cc, half * 384 : (half + 1) * 384],
                        start=(step == 0),
                        stop=(step == NC - 1),
                    )
                nc.vector.tensor_copy(
                    vt[sq][:, half * 6 : (half + 1) * 6, 0:D],
                    ps[:].rearrange("p (h d) -> p h d", d=D),
                )

            def head_scores(h):
                # A = [kc0 (q 0:128) | kc2 (q 0:384)], B = [kc1 (q 0:256)]
                # B runs first so its exp clears the PSUM ring early.
                # Causal masking happens post-exp as a 0/1 multiply on the
                # SBUF e-tiles (exp(-1e4)==0 == exp(s)*0), which keeps the
                # PSUM->exp chain short and runs on the idle engines.
                oc, prow = h // 2, (h % 2) * D
                Qh = q_sb[oc][prow : prow + D, :]
                Kh = k_sb[oc][prow : prow + D, :]
                ps_b = psS.tile([128, 512], f32, tag="s", name="sb")
                ps_a = psS.tile([128, 512], f32, tag="s", name="sa")
                nc.tensor.matmul(
                    ps_b[:, 0:256], Kh[:, 128:256], Qh[:, 0:256],
                    start=True, stop=True,
                )
                eB = epool.tile([128, 256], bf16, tag="eB", name="eB")
                nc.scalar.activation(eB[:], ps_b[:, 0:256], Exp)
                nc.gpsimd.tensor_mul(eB[:, 128:256], eB[:, 128:256], mb[:])
                nc.tensor.matmul(
                    ps_a[:, 0:128], Kh[:, 0:128], Qh[:, 0:128],
                    start=True, stop=True,
                )
                nc.tensor.matmul(
                    ps_a[:, 128:512], Kh[:, 256:384], Qh[:, 0:384],
                    start=True, stop=True, skip_group_check=True,
                )
                eA = epool.tile([128, 512], bf16, tag="eA", name="eA")
                nc.scalar.activation(eA[:], ps_a[:], Exp)
                # one strided op masks both diagonal blocks (cols 0:128
                # and 384:512 = stride-3 pairs of 128-col groups)
                eAv = eA[:].rearrange("p (a b) -> p a b", b=128)[:, ::3, :]
                _, mbb = bass.broadcast_tensor_aps(
                    eAv, mb.rearrange("p (a b) -> p a b", a=1)
                )
                nc.gpsimd.tensor_mul(eAv, eAv, mbb)
                return eA, eB

            def head_av(h, eA, eB):
                # attn@V with fused denominator column; accumulate widest
                # first so every element's first write carries start
                ps_av = psV.tile([D + 1, S], f32, tag="av", name="av")
                nc.tensor.matmul(
                    ps_av[:, 0:384], vt[2][:, h, :], eA[:, 128:512],
                    start=True, stop=False, skip_group_check=True,
                )
                nc.tensor.matmul(
                    ps_av[:, 0:256], vt[1][:, h, :], eB[:, 0:256],
                    start=False, stop=False, skip_group_check=True,
                )
                nc.tensor.matmul(
                    ps_av[:, 0:128], vt[0][:, h, :], eA[:, 0:128],
                    start=False, stop=True, skip_group_check=True,
                )
                nc.scalar.copy(au[:, h, :], ps_av[:])

            # normalize groups: heads (0-3), (4-7), (8-9), (10-11) at
            # partition bases 0/32/64/96 (compute ops need 32-aligned
            # partition starts)
            GRP = [range(0, 4), range(4, 8), range(8, 10), range(10, 12)]

            def norm_pre(g):
                # one DMA scatters the whole group's denominator rows to
                # [8 partitions x 48] blocks, then a full-partition
                # reciprocal and one repack DMA feed the broadcasts
                hs = GRP[g]
                p0, np_ = 32 * g, 8 * len(hs)
                nc.gpsimd.dma_start(
                    sums_sb[p0 : p0 + np_, :],
                    au[D : D + 1, hs[0] : hs[0] + len(hs), :],
                )
                with nc.allow_low_precision(
                    reason="1/softmax-denom in bf16: denom is O(1-20), "
                    "bf16 keeps ~0.4% relative error, well within budget"
                ):
                    nc.vector.reciprocal(
                        rinv_sb[p0 : p0 + np_, :], sums_sb[p0 : p0 + np_, :]
                    )
                nc.gpsimd.dma_start(
                    rinv_r[0:1, hs[0] : hs[0] + len(hs), :],
                    rinv_sb[p0 : p0 + np_, :],
                )

            def norm_bcast(h):
                # K=1 matmul broadcasts 1/sum across 64 partitions, then
                # the DVE multiply writes the normalized bf16 attn chunk
                oc, prow = h // 2, (h % 2) * D
                ps_r = psR.tile([D, S], f32, tag="rb", name="rb")
                nc.tensor.matmul(
                    ps_r[:], ones_sb[:], rinv_r[0:1, h, :],
                    start=True, stop=True,
                )
                nc.vector.tensor_mul(
                    attn_sb[oc][prow : prow + D, :], au[0:D, h, :], ps_r[:]
                )

            # ---- schedule ---------------------------------------------
            # Q/K projections and scores run ahead while wv streams in;
            # attn@V starts once the V projection lands. Remaining weight
            # DMAs issue from mid-program so the Activation engine's FIFO
            # stays responsive for the proj-copy/exp pipeline. Normalize
            # broadcasts interleave with the next chunk's matmuls to hide
            # the denominator chain (copy->DMA->recip->DMA) latency.
            # psP is single-buffered: q/k projections alternate with
            # scores so the ring never waits on its evacuation copy.
            # psV likewise gets ~1us of independent PE work between
            # attn@V calls.
            # ~35 throwaway matmuls during the otherwise-idle DMA wait
            # warm the PE's power state (it needs ~3us of continuous
            # execution to reach full speed; cold it runs ~1.5x slow for
            # the first dozen real matmuls). They only need the ones
            # vector, finish before x/wq0 land, and each bcast later
            # overwrites the scratch bank with start=True.
            for _ in range(35):
                ps_w = psR.tile([D, S], f32, tag="rb", name="warm")
                nc.tensor.matmul(
                    ps_w[:, 0:D], ones_sb[:], ones_sb[:],
                    start=True, stop=True,
                )

            es = {}
            qk_proj(0, wqb, 0, q_sb[0], True)
            nc.scalar.dma_start(wqb[1][:], wq_d[1])
            qk_proj(0, wkb, 6, k_sb[0], False)
            es[0] = head_scores(0)
            nc.scalar.dma_start(
                wqall[:, 2:4], wq_d[2:4].rearrange("o p c -> p o c")
            )
            es[1] = head_scores(1)
            qk_proj(1, wqb, 1, q_sb[1], True)
            nc.scalar.dma_start(wvb[:, 0:3, :], wv_d[:, 0 : 3 * C])
            qk_proj(1, wkb, 7, k_sb[1], False)
            nc.scalar.dma_start(
                wqall[:, 4:6], wq_d[4:6].rearrange("o p c -> p o c")
            )
            es[2] = head_scores(2)
            es[3] = head_scores(3)
            qk_proj(2, wqb, 2, q_sb[2], True)
            qk_proj(2, wkb, 8, k_sb[2], False)
            es[4] = head_scores(4)
            es[5] = head_scores(5)
            for sq in range(3):
                v_proj_unit(0, sq)
            for sq in range(3):
                v_proj_unit(1, sq)
            # psV is single-buffered: each attn@V is followed by ~1us of
            # independent PE work so the next one never waits on the
            # PSUM evacuation copy
            head_av(0, *es.pop(0))
            qk_proj(3, wqb, 3, q_sb[3], True)
            head_av(1, *es.pop(1))
            qk_proj(3, wkb, 9, k_sb[3], False)
            head_av(2, *es.pop(2))
            es[6] = head_scores(6)
            head_av(3, *es.pop(3))
            es[7] = head_scores(7)
            norm_pre(0)
            head_av(4, *es.pop(4))
            qk_proj(4, wqb, 4, q_sb[4], True)
            head_av(5, *es.pop(5))
            qk_proj(4, wkb, 10, k_sb[4], False)
            norm_bcast(0)
            norm_bcast(1)
            es[8] = head_scores(8)
            norm_bcast(2)
            norm_bcast(3)
            es[9] = head_scores(9)
            head_av(6, *es.pop(6))
            qk_proj(5, wqb, 5, q_sb[5], True)
            head_av(7, *es.pop(7))
            qk_proj(5, wkb, 11, k_sb[5], False)
            # ---- output projection (bias = host-folded Wo @ bv) -------
            # Partial accumulations over cc 0..3 (which need only the
            # already-normalized heads 0-7) fill the PE gaps where the
            # h8-11 exp and normalize chains would otherwise stall it;
            # chunks 2-4 borrow the (now idle) scores PSUM ring. The
            # cc=4,5 steps follow the group-2/3 multiplies.
            def out_proj_mm(ps, oc, ccs, start, stop):
                for i, cc in enumerate(ccs):
                    nc.tensor.matmul(
                        ps[:],
                        wob[oc][:, cc, :],
                        attn_sb[cc][:],
                        start=(start and i == 0),
                        stop=(stop and i == len(ccs) - 1),
                    )

            def out_proj_fin(ps, oc):
                ot = opool.tile([128, S], bf16, tag="o", name="o")
                nc.scalar.activation(
                    ot[:], ps[:], Ident, bias=cb[:, 12 + oc : 13 + oc]
                )
                eng = nc.sync if oc % 2 == 0 else nc.scalar
                eng.dma_start(y_d[oc * 128 : (oc + 1) * 128, :], ot[:])

            norm_pre(1)
            head_av(8, *es.pop(8))
            es[10] = head_scores(10)
            head_av(9, *es.pop(9))
            es[11] = head_scores(11)
            norm_pre(2)
            ps0 = psP.tile([128, S], f32, tag="proj", name="op")
            out_proj_mm(ps0, 0, [0, 1], True, False)
            head_av(10, *es.pop(10))
            norm_bcast(4)
            norm_bcast(5)
            ps1 = psP.tile([128, S], f32, tag="proj", name="op")
            out_proj_mm(ps1, 1, [0, 1], True, False)
            head_av(11, *es.pop(11))
            norm_pre(3)
            norm_bcast(6)
            norm_bcast(7)
            out_proj_mm(ps0, 0, [2, 3], False, False)
            out_proj_mm(ps1, 1, [2, 3], False, False)
            ps2 = psS.tile([128, 512], f32, tag="s", name="op")[:, 0:S]
            out_proj_mm(ps2, 2, range(4), True, False)
            norm_bcast(8)
            norm_bcast(9)
            ps3 = psS.tile([128, 512], f32, tag="s", name="op")[:, 0:S]
            out_proj_mm(ps3, 3, range(4), True, False)
            norm_bcast(10)
            norm_bcast(11)
            ps4 = psS.tile([128, 512], f32, tag="s", name="op")[:, 0:S]
            out_proj_mm(ps4, 4, range(4), True, False)
            # all cc=4 steps (gated only by group 2's multiplies) run
            # before any cc=5 step so the group-3 multiply latency is
            # hidden behind them
            chunks = ((0, ps0), (1, ps1), (2, ps2), (3, ps3), (4, ps4))
            for oc, ps in chunks:
                out_proj_mm(ps, oc, [4], False, False)
            for oc, ps in chunks:
                out_proj_mm(ps, oc, [5], False, True)
                out_proj_fin(ps, oc)
            ps5 = psP.tile([128, S], f32, tag="proj", name="op")
            out_proj_mm(ps5, 5, range(NC), True, True)
            out_proj_fin(ps5, 5)

    return nc


def _get_nc():
    if "nc" not in _STATE:
        _STATE["nc"] = _build_nc()
    return _STATE["nc"]


# --------------------------------------------------------------------------
def _prep_maps(inputs):
    import ml_dtypes

    bf16 = ml_dtypes.bfloat16
    hs = np.asarray(inputs["hidden_states"], dtype=np.float32)
    Wq = np.asarray(inputs["Wq"], dtype=np.float32)
    bq = np.asarray(inputs["bq"], dtype=np.float32)
    Wk = np.asarray(inputs["Wk"], dtype=np.float32)
    bk = np.asarray(inputs["bk"], dtype=np.float32)
    Wv = np.asarray(inputs["Wv"], dtype=np.float32)
    bv = np.asarray(inputs["bv"], dtype=np.float32)
    Wo = np.asarray(inputs["Wo"], dtype=np.float32)

    # head-major channel permutation: c' = h*64 + d  <-  c = d*12 + h
    idx = (np.arange(H)[:, None] + np.arange(D)[None, :] * H).reshape(C)
    scale = float(D) ** -0.5

    def pack_blocks(wt):
        # wt: [c_in, c_out] -> [oc, p, cc*128 + co]
        w4 = wt.reshape(NC, 128, NC, 128).transpose(2, 1, 0, 3)
        return np.ascontiguousarray(w4.reshape(NC, 128, C)).astype(bf16)

    wqp = pack_blocks((scale * Wq[idx, :]).T)
    wkp = pack_blocks(Wk[idx, :].T)
    wop = pack_blocks(Wo.T)
    # wv: [p, cc*768 + co]
    wvp = np.ascontiguousarray(
        Wv[idx, :].T.reshape(NC, 128, C).transpose(1, 0, 2).reshape(128, NC * C)
    ).astype(bf16)

    cbm = np.zeros((128, 82), dtype=np.float32)
    cbm[:, 0:6] = (scale * bq[idx]).reshape(6, 128).T
    cbm[:, 6:12] = bk[idx].reshape(6, 128).T
    # V-bias folded through attention (softmax rows sum to 1):
    # attn' = attn_nobias' + bv[idx], so out += Wo @ bv[idx]
    cbm[:, 12:18] = (Wo @ bv[idx]).reshape(6, 128).T
    # post-exp causal mask for a diagonal 128-block (keep k >= q),
    # bf16 0/1 values bitcast into the f32 consts tensor
    mbm = np.tril(np.ones((128, 128), dtype=np.float32)).astype(bf16)
    cbm[:, 18:82] = np.ascontiguousarray(mbm).view(np.float32)

    shared = {"wqp": wqp, "wkp": wkp, "wop": wop, "wvp": wvp, "cb": cbm}
    maps = []
    for b in range(B):
        xb = hs[b, :, 0, :].reshape(NC, 128, S).transpose(1, 0, 2)
        xp = np.ascontiguousarray(xb.reshape(128, NC * S)).astype(bf16)
        maps.append({"xp": xp, **shared})
    return maps


def _run(inputs, trace=False, **kwargs):
    from concourse.bass_utils import run_bass_kernel_spmd

    nc = _get_nc()
    in_maps = _prep_maps(inputs)
    res = run_bass_kernel_spmd(
        nc, in_maps, core_ids=list(range(B)), trace=trace, **kwargs
    )
    out = np.stack(
        [res.results[b]["y"].astype(np.float32) for b in range(B)], axis=0
    )
    return out.reshape(B, C, 1, S), res


def kernel(**inputs):
    out, _ = _run(inputs, trace=False)
    return out
